# revision 1
# baseline (speedup 1.0000x reference)
import sys

for p in ("/opt/trn_rl_repo",):
    if p not in sys.path:
        sys.path.insert(0, p)

import numpy as np
import ml_dtypes

from concourse import bass, mybir, bacc, tile
from concourse.ap import AP
from concourse.bass_utils import run_bass_kernel_spmd


def _install_ntff_hook():
    try:
        from antenv import axon_hooks  # noqa: F401
        return
    except ImportError:
        pass
    import types
    try:
        import antenv
    except ImportError:
        return
    mod = types.ModuleType("antenv.axon_hooks")
    _h = {"hook": None}
    mod.set_axon_ntff_profile_hook = lambda h: _h.__setitem__("hook", h)
    mod.get_axon_ntff_profile_hook = lambda: _h["hook"]
    sys.modules["antenv.axon_hooks"] = mod
    antenv.axon_hooks = mod
    try:
        from trn_agent_boot.trn_boot import _ntff_profile_via_ctypes
        h = _ntff_profile_via_ctypes("/opt/axon/libaxon_pjrt.so")
        if h is not None:
            mod.set_axon_ntff_profile_hook(h)
    except Exception:
        pass


_install_ntff_hook()


def _enable_ldw_opt():
    """walrus --enable-ldw-opt=false is hardcoded; flip it so LDWEIGHTS
    double-buffers against in-flight matmuls (PE was fully serialized)."""
    import concourse.bass_utils as _bu
    if getattr(_bu, "_ldw_patched", False):
        return
    _orig = _bu.run_command

    def _patched(argv, **kw):
        try:
            argv = ["--enable-ldw-opt=true" if c == "--enable-ldw-opt=false" else c
                    for c in argv]
        except TypeError:
            pass
        return _orig(argv, **kw)

    _bu.run_command = _patched
    _bu._ldw_patched = True


F32 = mybir.dt.float32
F32R = mybir.dt.float32r
BF16 = mybir.dt.bfloat16
MUL = mybir.AluOpType.mult
ADD = mybir.AluOpType.add
MAX = mybir.AluOpType.max
AXX = mybir.AxisListType.X
EXP = mybir.ActivationFunctionType.Exp

B, C, H, W = 16, 256, 96, 96
CQ = 32
S = H * W          # 9216
NB = 32            # bands per direction
NCORE = 8
BPC = B // NCORE   # 2 batches per core
QKW = 72           # q(32) | k(32) | sigma(1) | pad(7)
PW = QKW + 256     # 328 proj width


def _apv(t, off, dims):
    """Custom view on a tile/tensor AP: keep partition dim, custom free dims."""
    b = t[:] if not isinstance(t, AP) else t
    part = list(b.ap[0])
    return AP(b.tensor, b.offset + off, [part] + [list(d) for d in dims])


def build_graph():
    nc = bacc.Bacc(None, target_bir_lowering=False)

    xa_e = nc.declare_dram_parameter("xa", [BPC, 2, 128, S], BF16, isOutput=False)
    wall_e = nc.declare_dram_parameter("wall", [2, 128, PW], BF16, isOutput=False)
    ipat_e = nc.declare_dram_parameter("ipat", [96, 864], BF16, isOutput=False)
    pstr_e = nc.declare_dram_parameter("pstr", [96, 96], BF16, isOutput=False)
    idt_e = nc.declare_dram_parameter("idt", [96, 96], F32, isOutput=False)
    idtb_e = nc.declare_dram_parameter("idtb", [96, 96], BF16, isOutput=False)
    gam_e = nc.declare_dram_parameter("gam", [128, 1], F32, isOutput=False)
    bvrow_e = nc.declare_dram_parameter("bvrow", [1, 96 * 256], BF16, isOutput=False)
    out_e = nc.declare_dram_parameter("out", [BPC, 2, 128, S], BF16, isOutput=True)

    NCH = 65  # q32 | k32 | sigma

    with tile.TileContext(nc) as tc:
        with (
            tc.tile_pool(name="const", bufs=1) as cp,
            tc.tile_pool(name="main", bufs=1) as mp,
            tc.tile_pool(name="work", bufs=2) as wp,
            tc.tile_pool(name="pj", bufs=2, space="PSUM") as pj,
            tc.tile_pool(name="avp", bufs=2, space="PSUM") as avp,
        ):
            wall_sb = []
            for cc in range(2):
                t = cp.tile([128, PW], BF16, tag=f"wall{cc}")
                nc.sync.dma_start(t[:], wall_e[cc])
                wall_sb.append(t)
            ipat_sb = cp.tile([96, 864], BF16, tag="ipat")
            nc.sync.dma_start(ipat_sb[:], ipat_e[:])
            pstr_sb = cp.tile([96, 96], BF16, tag="pstr")
            nc.sync.dma_start(pstr_sb[:], pstr_e[:])
            idt_sb = cp.tile([96, 96], F32, tag="idt")
            nc.sync.dma_start(idt_sb[:], idt_e[:])
            idtb_sb = cp.tile([96, 96], BF16, tag="idtb")
            nc.sync.dma_start(idtb_sb[:], idtb_e[:])
            gam_sb = cp.tile([128, 1], F32, tag="gam")
            nc.sync.dma_start(gam_sb[:], gam_e[:])

            # v rows 0..95 = value projection (per line); row 96 = bv replicated
            v_sb = mp.tile([97, 96 * 256], BF16, tag="v")
            nc.sync.dma_start(v_sb[96:97, :], bvrow_e[:])

            NCH = 65
            st = {}

            def stage_load(b):
                xs = []
                for cc in range(2):
                    t = mp.tile([128, S], BF16, tag=f"xa{cc}", bufs=2,
                                name=f"xa{cc}_{b}")
                    nc.sync.dma_start(t[:], xa_e[b, cc])
                    xs.append(t)
                st[b] = {"xa": xs}

            def stage_hproj_group(b, g):
                xs = st[b]["xa"]
                ps = pj.tile([96, 1024], F32, tag="pj", name=f"ps{b}_{g}")
                for l2 in range(2):
                    h = 2 * g + l2
                    o = 512 * l2
                    for cc in range(2):
                        nc.tensor.matmul(
                            _apv(ps, o, [[1, PW]]),
                            xs[cc][:, h * 96:(h + 1) * 96],
                            wall_sb[cc][:],
                            start=(cc == 0),
                            stop=(cc == 1),
                        )
                qk_sb = st[b]["qk"]
                nc.vector.tensor_copy(
                    qk_sb[:, g * 2 * QKW:(g + 1) * 2 * QKW],
                    _apv(ps, 0, [[512, 2], [1, QKW]]),
                )
                if g % 2 == 0:
                    nc.scalar.copy(
                        v_sb[0:96, g * 512:(g + 1) * 512],
                        _apv(ps, QKW, [[512, 2], [1, 256]]),
                    )
                else:
                    nc.vector.tensor_copy(
                        v_sb[0:96, g * 512:(g + 1) * 512],
                        _apv(ps, QKW, [[512, 2], [1, 256]]),
                    )

            def stage_transp(b):
                qk_sb, qkc_sb = st[b]["qk"], st[b]["qkc"]
                for grp in range(9):
                    nch = min(8, NCH - grp * 8)
                    ptq = pj.tile([96, 1024], BF16, tag="pj", name=f"ptq{b}_{grp}")
                    for i in range(nch):
                        ch = grp * 8 + i
                        nc.tensor.transpose(
                            ptq[:, i * 96:(i + 1) * 96],
                            _apv(qk_sb, ch, [[QKW, 96]]),
                            idtb_sb[:],
                        )
                    nc.vector.tensor_copy(
                        qkc_sb[:, grp * 768:grp * 768 + nch * 96],
                        ptq[:, 0:nch * 96],
                    )

            def stage_scores(b):
                qk_sb, qkc_sb = st[b]["qk"], st[b]["qkc"]
                sc = {}
                st[b]["sc"] = sc
                for nm in ("h", "v"):
                    for key, shp, dt_ in (("sraw", 288, F32), ("sx", 288, F32),
                                          ("m3", 96, F32), ("te", 288, F32),
                                          ("s3", 96, F32), ("r3", 96, F32),
                                          ("A", 288, BF16)):
                        sc[key + nm] = mp.tile([96, shp], dt_, tag=f"{key}{nm}",
                                               name=f"{key}{nm}{b}")
                a_h, a_v = sc["Ah"], sc["Av"]
                b_h = mp.tile([96, 96], F32, tag="bh", name=f"bh{b}")
                btot = mp.tile([96, 96], F32, tag="btot", name=f"btot{b}")

                def scores_part(nm, n0, nn, qv, kv, sv):
                    s_raw, sx = sc["sraw" + nm], sc["sx" + nm]
                    m3, te = sc["m3" + nm], sc["te" + nm]
                    s3, r3, a_t = sc["s3" + nm], sc["r3" + nm], sc["A" + nm]
                    for k in range(3):
                        for j in range(3):
                            prod = wp.tile([96, 32 * 16], BF16, tag="prod")
                            nc.vector.tensor_tensor(
                                _apv(prod, 0, [[1, 32 * nn]]),
                                qv(k, n0, nn), kv(j, n0, nn),
                                MUL,
                            )
                            nc.vector.tensor_reduce(
                                s_raw[:, (3 * k + j) * 32 + n0:(3 * k + j) * 32 + n0 + nn],
                                _apv(prod, 0, [[32, nn], [1, 32]]),
                                AXX, ADD,
                            )
                    nc.gpsimd.tensor_tensor(
                        _apv(sx, n0 * 3, [[96, 3], [1, 3 * nn]]),
                        _apv(s_raw, n0, [[96, 3], [1, nn], [32, 3]]),
                        sv(n0, nn),
                        ADD,
                    )
                    v_knj = lambda t: _apv(t, n0 * 3, [[96, 3], [3, nn], [1, 3]])
                    v_kn = lambda t: _apv(t, n0, [[32, 3], [1, nn]])
                    nc.vector.tensor_reduce(v_kn(m3), v_knj(sx), AXX, MAX)
                    nc.gpsimd.tensor_tensor(
                        v_knj(te), v_knj(sx),
                        _apv(m3, n0, [[32, 3], [1, nn], [0, 3]]),
                        mybir.AluOpType.subtract,
                    )
                    nc.scalar.activation(v_knj(te), v_knj(te), EXP)
                    nc.vector.tensor_reduce(v_kn(s3), v_knj(te), AXX, ADD)
                    nc.vector.reciprocal(v_kn(r3), v_kn(s3))
                    nc.gpsimd.tensor_tensor(
                        v_knj(a_t), v_knj(te),
                        _apv(r3, n0, [[32, 3], [1, nn], [0, 3]]),
                        MUL,
                    )

                qh = lambda k, n0, nn: _apv(qk_sb, k * QKW + n0 * 3 * QKW,
                                            [[3 * QKW, nn], [1, 32]])
                kh = lambda j, n0, nn: _apv(qk_sb, j * QKW + 32 + n0 * 3 * QKW,
                                            [[3 * QKW, nn], [1, 32]])
                sh = lambda n0, nn: _apv(qk_sb, 64 + n0 * 3 * QKW,
                                         [[0, 3], [3 * QKW, nn], [QKW, 3]])
                qv = lambda k, n0, nn: _apv(qkc_sb, k + n0 * 3,
                                            [[3, nn], [96, 32]])
                kv = lambda j, n0, nn: _apv(qkc_sb, 32 * 96 + j + n0 * 3,
                                            [[3, nn], [96, 32]])
                sv = lambda n0, nn: _apv(qkc_sb, 64 * 96 + n0 * 3,
                                         [[0, 3], [3, nn], [1, 3]])

                for half in range(2):
                    scores_part("v", half * 16, 16, qv, kv, sv)
                for half in range(2):
                    scores_part("h", half * 16, 16, qh, kh, sh)
                    nc.vector.tensor_reduce(
                        b_h[:, half * 48:(half + 1) * 48],
                        _apv(a_h, half * 48, [[3, 16], [1, 3], [96, 3]]), AXX, ADD)

                av_p = wp.tile([96, 288], F32, tag="avp")
                nc.vector.tensor_copy(
                    av_p[:], _apv(a_v, 0, [[1, 3], [3, 32], [96, 3]]))
                av_s = wp.tile([96, 96], F32, tag="avs")
                nc.vector.tensor_reduce(
                    av_s[:], _apv(a_v, 0, [[3, 32], [1, 3], [96, 3]]), AXX, ADD)

                avt = mp.tile([96, 288], BF16, tag="avt", name=f"avt{b}")
                avst = mp.tile([96, 96], F32, tag="avst", name=f"avst{b}")
                for jp in range(3):
                    pt = pj.tile([96, 96], F32, tag="pj", name=f"pt{b}_{jp}")
                    nc.tensor.transpose(
                        pt[:, 0:96], av_p[:, jp * 96:(jp + 1) * 96], idt_sb[:])
                    nc.vector.tensor_copy(avt[:, jp * 96:(jp + 1) * 96], pt[:, 0:96])
                pt = pj.tile([96, 96], F32, tag="pj", name=f"pts{b}")
                nc.tensor.transpose(pt[:, 0:96], av_s[:], idt_sb[:])
                nc.vector.tensor_copy(avst[:], pt[:, 0:96])

                nc.gpsimd.tensor_tensor(btot[:], b_h[:], avst[:], ADD)
                btb = mp.tile([96, 96], BF16, tag="btb", name=f"btb{b}")
                nc.vector.tensor_copy(btb[:], btot[:])
                bt = mp.tile([96, 96], BF16, tag="bt", name=f"bt{b}")
                ptb = pj.tile([96, 192], BF16, tag="pj", name=f"ptb{b}")
                nc.tensor.transpose(ptb[:, 0:96], btb[:], idtb_sb[:])
                nc.vector.tensor_copy(bt[:], ptb[:, 0:96])
                st[b]["avt"], st[b]["bt"] = avt, bt
                st[b]["mv"] = mp.tile([96, 9216], BF16, tag="mv", name=f"mv{b}")

            def stage_mv_quarter(b, q4):
                nc.gpsimd.tensor_tensor(
                    _apv(st[b]["mv"], q4 * 2304, [[96, 24], [3, 32], [1, 3]]),
                    _apv(pstr_sb, 0, [[0, 24], [3, 32], [1, 3]]),
                    _apv(st[b]["avt"], q4 * 24, [[1, 24], [0, 32], [96, 3]]),
                    MUL,
                )

            def stage_av_band(b, n):
                a_h, bt, mv = st[b]["sc"]["Ah"], st[b]["bt"], st[b]["mv"]
                xs = st[b]["xa"]
                rhs = wp.tile([97, 864], BF16, tag="rhs")
                beng = nc.vector if n % 2 == 0 else nc.gpsimd
                beng.tensor_tensor(
                    rhs[0:96, 0:864],
                    ipat_sb[:, 0:864],
                    _apv(a_h, n * 3, [[96, 3], [1, 3], [0, 96]]),
                    MUL,
                )
                nc.gpsimd.tensor_tensor(
                    AP(rhs[:].tensor, rhs[:].offset, [[864, 96], [384, 3], [1, 96]]),
                    AP(rhs[:].tensor, rhs[:].offset, [[864, 96], [384, 3], [1, 96]]),
                    _apv(mv, 3 * n * 96, [[96, 3], [1, 96]]),
                    ADD,
                )
                nc.sync.dma_start(rhs[96:97, 0:288], bt[3 * n:3 * n + 3, :])
                for cc in range(2):
                    pso = avp.tile([128, 288], F32, tag=f"av{cc}")
                    nc.tensor.matmul(
                        pso[:],
                        _apv(v_sb, (3 * n) * 256 + cc * 128, [[1, 128]]),
                        rhs[:, 0:288],
                        start=True, stop=False,
                    )
                    for k in (1, 2):
                        nc.tensor.matmul(
                            pso[:],
                            AP(v_sb[:].tensor,
                               v_sb[:].offset + (3 * n + k) * 256 + cc * 128,
                               [[96 * 256, 96], [1, 128]]),
                            rhs[0:96, k * 288:(k + 1) * 288],
                            start=False, stop=(k == 2),
                        )
                    nc.vector.scalar_tensor_tensor(
                        xs[cc][:, n * 288:(n + 1) * 288],
                        pso[:],
                        gam_sb[:],
                        xs[cc][:, n * 288:(n + 1) * 288],
                        MUL, ADD,
                    )
                    nc.sync.dma_start(
                        out_e[b, cc, :, n * 288:(n + 1) * 288],
                        xs[cc][:, n * 288:(n + 1) * 288],
                    )

            # ---- software-pipelined emission across the two batches ----
            stage_load(0)
            st[0]["qk"] = mp.tile([96, 96 * QKW], BF16, tag="qk", name="qk0")
            st[0]["qkc"] = mp.tile([96, NCH * 96], BF16, tag="qkc", name="qkc0")
            for g in range(48):
                stage_hproj_group(0, g)
            stage_transp(0)
            stage_scores(0)
            stage_mv_quarter(0, 0)
            stage_load(1)
            st[1]["qk"] = mp.tile([96, 96 * QKW], BF16, tag="qk", name="qk1")
            st[1]["qkc"] = mp.tile([96, NCH * 96], BF16, tag="qkc", name="qkc1")
            for n in range(8):
                stage_av_band(0, n)
            stage_mv_quarter(0, 1)
            g1 = 0
            for n in range(8, NB):
                stage_av_band(0, n)
                if n == 15:
                    stage_mv_quarter(0, 2)
                if n == 23:
                    stage_mv_quarter(0, 3)
                while g1 < 48 and g1 <= 2 * (n - 8):
                    stage_hproj_group(1, g1)
                    g1 += 1
            while g1 < 48:
                stage_hproj_group(1, g1)
                g1 += 1
            stage_transp(1)
            stage_scores(1)
            for q4 in range(4):
                stage_mv_quarter(1, q4)
            for n in range(NB):
                stage_av_band(1, n)
    nc.compile()
    return nc


def _host_prep(x, Wq, bq, Wk, bk, Wv, bv, gamma):
    x = np.ascontiguousarray(x, np.float32)
    sig_w = (bq @ Wk).astype(np.float32)          # [256]
    pad = np.zeros((7, 256), np.float32)
    wall = np.concatenate([Wq, Wk, sig_w[None], pad, Wv], 0)      # [328, 256]
    wallT = np.stack([np.ascontiguousarray(wall[:, :128].T),
                      np.ascontiguousarray(wall[:, 128:].T)])     # [2,128,328]
    ipat = np.tile(np.eye(96), (1, 9)).astype(ml_dtypes.bfloat16)   # [96, 864]
    pstr = np.kron(np.eye(32), np.ones((3, 3))).astype(ml_dtypes.bfloat16)
    idt = np.eye(96, dtype=np.float32)
    idtb = np.eye(96).astype(ml_dtypes.bfloat16)
    gam = np.full((128, 1), float(np.asarray(gamma).reshape(-1)[0]), np.float32)
    bvrow = np.tile(bv.astype(np.float32), 96)[None, :].astype(ml_dtypes.bfloat16)
    xr = x.reshape(B, 2, 128, S)
    in_maps = []
    for i in range(NCORE):
        in_maps.append({
            "xa": np.ascontiguousarray(xr[i * BPC:(i + 1) * BPC]).astype(ml_dtypes.bfloat16),
            "wall": wallT.astype(ml_dtypes.bfloat16),
            "ipat": ipat, "pstr": pstr, "idt": idt, "idtb": idtb,
            "gam": gam, "bvrow": bvrow,
        })
    return in_maps


_CACHE = {}


def kernel(x, Wq, bq, Wk, bk, Wv, bv, gamma, _trace=False):
    x = np.asarray(x, np.float32)
    in_maps = _host_prep(x, np.asarray(Wq, np.float32), np.asarray(bq, np.float32),
                         np.asarray(Wk, np.float32), np.asarray(bk, np.float32),
                         np.asarray(Wv, np.float32), np.asarray(bv, np.float32),
                         np.asarray(gamma, np.float32))
    if "nc" not in _CACHE:
        _CACHE["nc"] = build_graph()
    nc = _CACHE["nc"]
    res = run_bass_kernel_spmd(nc, in_maps, list(range(NCORE)), trace=_trace)
    kernel.last_result = res
    out = np.empty((B, C, H, W), np.float32)
    for i in range(NCORE):
        o = np.asarray(res.results[i]["out"], np.float32)   # [BPC, 2, 128, S]
        for b in range(BPC):
            out[i * BPC + b] = o[b].reshape(C, H, W)
    return out


if __name__ == "__main__":
    rng = np.random.default_rng(0)
    xs = {k: rng.standard_normal(s).astype(np.float32) * (0.05 if k != "x" else 1.0)
          for k, s in [("x", (16, 256, 96, 96)), ("Wq", (32, 256)), ("bq", (32,)),
                       ("Wk", (32, 256)), ("bk", (32,)), ("Wv", (256, 256)),
                       ("bv", (256,)), ("gamma", (1,))]}
    y = kernel(**xs)
    print("ran", y.shape)

